# revision 8
# baseline (speedup 1.0000x reference)
"""Differentiable 2D log-chroma histogram on 8 Trainium2 NeuronCores.

Problem: img [4, 3, 384, 512] f32 -> out [4, 64, 64] f32 where
  u = ln(g+eps) - ln(r+eps), v = ln(g+eps) - ln(b+eps)
  Iy = sqrt(r^2+g^2+b^2) * (r+g+b > eps)
  N[b,j,i] = sum_p Iy * (0<|v - A_v[j]|<=eps_bin) * (0<|u - A_u[i]|<=eps_bin)
  out = sqrt((N+1e-8) / (sum(N+1e-8)+1e-8))

Device algorithm (per core; batch b = core//2, height-half = core%2):
  Each pixel lands in exactly 2 consecutive u-bins {k, k+1} (k = floor((u-LO)/eps))
  and 2 consecutive v-bins, so the double-hot histogram N equals a 2x2 box-sum of
  the single-hot histogram H[j', i'] (j' = k_v+1, i' = k_u+1; width 66 = 65 live
  + 1 dead column; out-of-range indices match no one-hot column and drop out).

  DVE cost is the critical path (tensor_tensor is capped at 2 elem/cyc), so the
  v-side weighted one-hot wv = iy*onehot66(jv) is built FACTORED: jv = 6a + b,
  wv[p, 6a+b] = onehot11(a)[a] * (iy*onehot6(b))[b].  Per pixel that is
  22 + 6 + 6 + 66 = 100 mask elements instead of 66 + 66 + 66 = 198 for
  {eq, eq, mult} direct.  The u-side one-hot stays direct (66).  All index and
  weight operands are stored as bf16 *pairs* (each value duplicated in adjacent
  columns) so broadcast access patterns keep innermost step=1 and the DVE runs
  in 2x_1P packed mode; onehot11 is built pair-duplicated (vs iota 0,0,1,1,..)
  so the combine can broadcast it over the b-dim with innermost step=1.
  H is accumulated on the tensor engine: per 128-pixel tile, H += wv^T @ mu
  into one PSUM bank across all 768 tiles (PE runs ~32ns/tile when unblocked,
  far below DVE cadence).  Host folds H (2x2 box sum), combines core pairs,
  normalizes, sqrts.
"""

import os

import numpy as np

import concourse.bacc as bacc
import concourse.tile as tile
from concourse import mybir
from concourse.bass_utils import run_bass_kernel_spmd

NBINS = 64
HIST_LO, HIST_HI = -2.85, 2.85
EPS_BIN = (HIST_HI - HIST_LO) / (NBINS - 1)
EPS = 1e-8
P = 128
T = 768  # 128*768 = 98304 pixels per core = half of one batch image
NB = 66  # one-hot width: k+1 in [0, 64] + 1 dead column (= 11*6)
NA = 11  # outer digit: j' = 6*a + b
NBB = 6  # inner digit
TC = 64  # max tiles per mask chunk
CHUNK_SIZES = [64] * 11 + [44, 20]  # split last chunk -> short serial PE tail
CHUNK_STARTS = [sum(CHUNK_SIZES[:i]) for i in range(len(CHUNK_SIZES))]
MAGIC = 2.0**23  # f32 round-to-nearest-int via (x + 2^23) - 2^23

f32 = mybir.dt.float32
bf16 = mybir.dt.bfloat16
Act = mybir.ActivationFunctionType
Alu = mybir.AluOpType

_cache = {}


def _build_bass():
    nc = bacc.Bacc("TRN2", target_bir_lowering=False, debug=False, num_devices=8)
    rgb = nc.declare_dram_parameter("rgb", [3, P, T], f32, isOutput=False)
    hist = nc.declare_dram_parameter("hist", [NB, NB], f32, isOutput=True)

    gps_chunks = {
        int(x) for x in os.environ.get("HIST_GPS_CHUNKS", "").split(",") if x != ""
    }

    with tile.TileContext(nc) as tc:
        with (
            tc.tile_pool(name="const", bufs=1) as cpool,
            tc.tile_pool(name="px", bufs=1) as px,
            tc.tile_pool(name="mask", bufs=3) as mpool,
            tc.tile_pool(name="psum", bufs=1, space="PSUM") as pp,
        ):
            r = px.tile([P, T], f32, tag="r")
            g = px.tile([P, T], f32, tag="g")
            b = px.tile([P, T], f32, tag="b")
            # r and g gate the first mask op; load them first, split for
            # queue parallelism, b last.
            H2 = T // 2
            nc.sync.dma_start(r[:, 0:H2], rgb[0, :, 0:H2])
            nc.sync.dma_start(r[:, H2:T], rgb[0, :, H2:T])
            nc.sync.dma_start(g[:, 0:H2], rgb[1, :, 0:H2])
            nc.sync.dma_start(g[:, H2:T], rgb[1, :, H2:T])
            nc.sync.dma_start(b[:, 0:H2], rgb[2, :, 0:H2])
            nc.sync.dma_start(b[:, H2:T], rgb[2, :, H2:T])

            # tiny iotas; broadcast over tiles via stride-0 dims at use sites
            iota66 = cpool.tile([P, NB], bf16, tag="iota66")
            nc.gpsimd.iota(
                iota66[:], pattern=[[1, NB]], base=0,
                channel_multiplier=0, allow_small_or_imprecise_dtypes=True,
            )
            iota11p = cpool.tile([P, 2 * NA], bf16, tag="iota11p")
            nc.gpsimd.iota(
                iota11p[:], pattern=[[1, NA], [0, 2]], base=0,
                channel_multiplier=0, allow_small_or_imprecise_dtypes=True,
            )
            iota6 = cpool.tile([P, NBB], bf16, tag="iota6")
            nc.gpsimd.iota(
                iota6[:], pattern=[[1, NBB]], base=0,
                channel_multiplier=0, allow_small_or_imprecise_dtypes=True,
            )

            # Pre-touch each DMA-produced tile with a single-input op so the
            # DVE's vector clock observes each DMA semaphore once; later
            # multi-input ops then need fewer waits.
            warm = cpool.tile([P, 4], f32, tag="warm")
            nc.vector.tensor_copy(warm[:, 0:1], r[:, 0:1])
            nc.vector.tensor_copy(warm[:, 1:2], g[:, 0:1])
            nc.vector.tensor_copy(warm[:, 2:3], b[:, 0:1])

            eps_bias = cpool.tile([P, 1], f32, tag="eps_bias")
            nc.vector.memset(eps_bias[:], EPS)
            # tiny dummy Ln preloads the ACT table before the DMA completes
            tbl_warm = cpool.tile([P, 1], f32, tag="tbl_warm")
            nc.scalar.activation(tbl_warm[:], eps_bias[:], Act.Ln, bias=eps_bias[:])
            lr = px.tile([P, T], f32, tag="lr")
            lg = px.tile([P, T], f32, tag="lg")
            lb = px.tile([P, T], f32, tag="lb")
            nc.scalar.activation(lr[:], r[:], Act.Ln, bias=eps_bias[:])
            nc.scalar.activation(lg[:], g[:], Act.Ln, bias=eps_bias[:])
            nc.scalar.activation(lb[:], b[:], Act.Ln, bias=eps_bias[:])

            u = px.tile([P, T], f32, tag="u")
            v = px.tile([P, T], f32, tag="v")
            nc.vector.tensor_tensor(u[:], lg[:], lr[:], op=Alu.subtract)
            nc.vector.tensor_tensor(v[:], lg[:], lb[:], op=Alu.subtract)

            # iu+MAGIC = u/eps_bin + (0.5 - LO/eps_bin) + 2^23: the magic add
            # rounds to integer; the -2^23 is subtracted exactly by the ACT
            # pair-copy bias below.
            iu = px.tile([P, T], f32, tag="iu")
            jvm = px.tile([P, T], f32, tag="jvm")
            nc.vector.tensor_scalar(
                iu[:], u[:], 1.0 / EPS_BIN, 0.5 - HIST_LO / EPS_BIN + MAGIC,
                op0=Alu.mult, op1=Alu.add,
            )
            nc.vector.tensor_scalar(
                jvm[:], v[:], -1.0 / EPS_BIN, 0.5 + HIST_HI / EPS_BIN + MAGIC,
                op0=Alu.mult, op1=Alu.add,
            )
            # jv digits: jv = 6a + b.  jvs = exact small integer;
            # a = round(jv/6 - 5/12 + MAGIC) = floor(jv/6) for integer jv
            # (margins to the .5 rounding boundary are >= 1/12, product error
            # ~2^-24*|jv|/6 is negligible).  b = jv - 6a computed on de-magic'd
            # values only: anything ~6*2^23 has ulp 4 and would corrupt b.
            jvs = px.tile([P, T], f32, tag="jvs")
            nc.vector.tensor_scalar(
                jvs[:], jvm[:], 1.0, -MAGIC, op0=Alu.mult, op1=Alu.add
            )
            # MAGIC - 2.5/6 is NOT representable in f32 (ulp near 2^23 is 0.5,
            # the offset would collapse to -0.5 and round-half-even would then
            # misplace every b=0/a-odd pixel); apply the small offset first.
            a1 = px.tile([P, T], f32, tag="a1")
            nc.vector.tensor_scalar(
                a1[:], jvs[:], 1.0 / 6.0, -2.5 / 6.0, op0=Alu.mult, op1=Alu.add
            )
            am = px.tile([P, T], f32, tag="am")
            nc.vector.tensor_scalar(
                am[:], a1[:], 1.0, MAGIC, op0=Alu.mult, op1=Alu.add
            )
            asm = px.tile([P, T], f32, tag="asm")
            nc.vector.tensor_scalar(
                asm[:], am[:], 1.0, -MAGIC, op0=Alu.mult, op1=Alu.add
            )
            bsm = px.tile([P, T], f32, tag="bsm")
            nc.vector.scalar_tensor_tensor(
                bsm[:], asm[:], -6.0, jvs[:], op0=Alu.mult, op1=Alu.add
            )
            negM = cpool.tile([P, 1], f32, tag="negM")
            nc.vector.memset(negM[:], -MAGIC)
            zbias = cpool.tile([P, 1], f32, tag="zbias")
            nc.vector.memset(zbias[:], 0.0)

            # Iy = sqrt(r^2+g^2+b^2) * (r+g+b > EPS)
            r2 = px.tile([P, T], f32, tag="r2")
            g2 = px.tile([P, T], f32, tag="g2")
            b2 = px.tile([P, T], f32, tag="b2")
            nc.scalar.activation(r2[:], r[:], Act.Square)
            nc.scalar.activation(g2[:], g[:], Act.Square)
            nc.scalar.activation(b2[:], b[:], Act.Square)
            ss = px.tile([P, T], f32, tag="ss")
            nc.vector.tensor_tensor(ss[:], r2[:], g2[:], op=Alu.add)
            nc.vector.tensor_tensor(ss[:], ss[:], b2[:], op=Alu.add)
            # valid = (r+g+b > 1e-8) is omitted: with uniform [0,1) inputs the
            # probability of a pixel failing it is ~1e-24, and even then the
            # histogram perturbation would be ~1e-8 of one cell.

            # bf16 pair copies (value duplicated in adjacent columns).
            # Chunk-0 slices go into dedicated small tiles FIRST so the first
            # mask ops can issue before the full-width prep completes.
            def pair_full(name, src, bias, act=Act.Identity):
                pt0 = cpool.tile([P, 2 * TC], bf16, tag=name + "0")
                pt = px.tile([P, 2 * T], bf16, tag=name)
                nc.scalar.activation(
                    pt0[:].rearrange("p (t two) -> p two t", two=2),
                    src[:, 0:TC].unsqueeze(1).to_broadcast([P, 2, TC]),
                    act,
                    bias=bias,
                )
                nc.scalar.activation(
                    pt[:, 2 * TC :].rearrange("p (t two) -> p two t", two=2),
                    src[:, TC:].unsqueeze(1).to_broadcast([P, 2, T - TC]),
                    act,
                    bias=bias,
                )
                return pt0, pt

            iu_p0, iu_p = pair_full("iu_p", iu, negM[:])
            a_p0, a_p = pair_full("a_p", am, negM[:])
            b_p0, b_p = pair_full("b_p", bsm, zbias[:])
            iy_p0, iy_p = pair_full("iy_p", ss, zbias[:], act=Act.Sqrt)

            pairs_full = {"iu": iu_p, "a": a_p, "b": b_p, "iy": iy_p}
            pairs_0 = {"iu": iu_p0, "a": a_p0, "b": b_p0, "iy": iy_p0}

            def pair_bcast(key, c, inner):
                st, sz = CHUNK_STARTS[c], CHUNK_SIZES[c]
                full = pairs_full[key]
                if c == 0:
                    sl = pairs_0[key][:]
                else:
                    sl = full[:, st * 2 : (st + sz) * 2]
                return (
                    sl.rearrange("p (t two) -> p t two", two=2)
                    .unsqueeze(2)
                    .to_broadcast([P, sz, inner, 2])
                )

            def iota_bcast(tl, sz, inner):
                return (
                    tl[:]
                    .rearrange("p (h two) -> p h two", two=2)
                    .unsqueeze(1)
                    .to_broadcast([P, sz, inner, 2])
                )

            direct_v = bool(int(os.environ.get("HIST_DIRECT", "0")))
            if direct_v:
                jv_p0, jv_p = pair_full("jv_p", jvm, negM[:])
                pairs_full["jv"] = jv_p
                pairs_0["jv"] = jv_p0

            hp = pp.tile([NB, NB], f32, tag="hp")
            for c, (cst, csz) in enumerate(zip(CHUNK_STARTS, CHUNK_SIZES)):
                mu = mpool.tile([P, TC * NB], bf16, tag="mu")
                da = mpool.tile([P, TC * 2 * NA], bf16, tag="da")
                wb = mpool.tile([P, TC * NBB], bf16, tag="wb")
                wv = mpool.tile([P, TC * NB], bf16, tag="wv")
                mu4 = mu[:, 0 : csz * NB].rearrange(
                    "p (t h two) -> p t h two", h=NB // 2, two=2
                )
                da4 = da[:, 0 : csz * 2 * NA].rearrange(
                    "p (t k two) -> p t k two", k=NA, two=2
                )
                wb4 = wb[:, 0 : csz * NBB].rearrange(
                    "p (t h two) -> p t h two", h=NBB // 2, two=2
                )
                nc.vector.tensor_tensor(
                    mu4, pair_bcast("iu", c, NB // 2), iota_bcast(iota66, csz, NB // 2),
                    op=Alu.is_equal,
                )
                if direct_v:
                    wv4 = wv[:, 0 : csz * NB].rearrange(
                        "p (t h two) -> p t h two", h=NB // 2, two=2
                    )
                    nc.vector.tensor_tensor(
                        wv4, pair_bcast("jv", c, NB // 2),
                        iota_bcast(iota66, csz, NB // 2), op=Alu.is_equal,
                    )
                    nc.vector.tensor_tensor(
                        wv4, wv4, pair_bcast("iy", c, NB // 2), op=Alu.mult
                    )
                else:
                    nc.vector.tensor_tensor(
                        da4, pair_bcast("a", c, NA), iota_bcast(iota11p, csz, NA),
                        op=Alu.is_equal,
                    )
                    nc.vector.tensor_tensor(
                        wb4, pair_bcast("b", c, NBB // 2),
                        iota_bcast(iota6, csz, NBB // 2), op=Alu.is_equal,
                    )
                    nc.vector.tensor_tensor(
                        wb4, wb4, pair_bcast("iy", c, NBB // 2), op=Alu.mult
                    )
                    # wv[p, t, a, h, two] = da[p, t, a(dup-pair)] * wb[p, t, (h,two)]
                    da_e = (
                        da[:, 0 : csz * 2 * NA]
                        .rearrange("p (t a two) -> p t a two", a=NA, two=2)
                        .unsqueeze(3)
                        .to_broadcast([P, csz, NA, NBB // 2, 2])
                    )
                    wb_e = (
                        wb[:, 0 : csz * NBB]
                        .rearrange("p (t h two) -> p t h two", h=NBB // 2, two=2)
                        .unsqueeze(2)
                        .to_broadcast([P, csz, NA, NBB // 2, 2])
                    )
                    wv5 = wv[:, 0 : csz * NB].rearrange(
                        "p (t a h two) -> p t a h two", a=NA, h=NBB // 2, two=2
                    )
                    eng = nc.gpsimd if c in gps_chunks else nc.vector
                    eng.tensor_tensor(wv5, da_e, wb_e, op=Alu.mult)
                for t in range(csz):
                    gt = cst + t
                    nc.tensor.matmul(
                        hp[:],
                        lhsT=wv[:, t * NB : (t + 1) * NB],
                        rhs=mu[:, t * NB : (t + 1) * NB],
                        start=(gt == 0),
                        stop=(gt == T - 1),
                    )

            hs = cpool.tile([NB, NB], f32, tag="hs")
            nc.scalar.activation(hs[:], hp[:], Act.Copy)
            nc.sync.dma_start(hist[:], hs[:])
    nc.compile()
    return nc


def kernel(img: np.ndarray) -> np.ndarray:
    B, C, H, W_ = img.shape
    assert (B, C, H, W_) == (4, 3, 384, 512)
    img = np.ascontiguousarray(np.asarray(img, dtype=np.float32))

    if "nc" not in _cache:
        _cache["nc"] = _build_bass()
    nc = _cache["nc"]

    in_maps = []
    for core in range(8):
        bb, half = divmod(core, 2)
        shard = img[bb, :, half * 192 : (half + 1) * 192, :].reshape(3, P, T)
        in_maps.append({"rgb": np.ascontiguousarray(shard)})

    trace = bool(int(os.environ.get("HIST_TRACE", "0")))
    res = run_bass_kernel_spmd(nc, in_maps, list(range(8)), trace=trace)
    if trace:
        print(f"HW exec time: {res.exec_time_ns} ns")
        _cache["exec_time_ns"] = res.exec_time_ns

    out = np.empty((4, NBINS, NBINS), dtype=np.float32)
    for bb in range(4):
        h = res.results[2 * bb]["hist"].astype(np.float64) + res.results[
            2 * bb + 1
        ]["hist"].astype(np.float64)
        n = (
            h[0:64, 0:64]
            + h[0:64, 1:65]
            + h[1:65, 0:64]
            + h[1:65, 1:65]
        ) + 1e-8
        norm = n.sum() + 1e-8
        out[bb] = np.sqrt(n / norm).astype(np.float32)
    return out


# revision 13
# speedup vs baseline: 1.1255x; 1.1255x over previous
"""Differentiable 2D log-chroma histogram on 8 Trainium2 NeuronCores.

Problem: img [4, 3, 384, 512] f32 -> out [4, 64, 64] f32 where
  u = ln(g+eps) - ln(r+eps), v = ln(g+eps) - ln(b+eps)
  Iy = sqrt(r^2+g^2+b^2) * (r+g+b > eps)
  N[b,j,i] = sum_p Iy * (0<|v - A_v[j]|<=eps_bin) * (0<|u - A_u[i]|<=eps_bin)
  out = sqrt((N+1e-8) / (sum(N+1e-8)+1e-8))

Device algorithm (per core; batch b = core//2, height-half = core%2):
  Each pixel lands in exactly 2 consecutive u-bins {k, k+1} (k = floor((u-LO)/eps))
  and 2 consecutive v-bins, so the double-hot histogram N equals a 2x2 box-sum of
  the single-hot histogram H[j', i'] (j' = k_v+1, i' = k_u+1; width 66 = 65 live
  + 1 dead column; out-of-range indices match no one-hot column and drop out).

  DVE cost is the critical path (tensor_tensor is capped at 2 elem/cyc), so the
  v-side weighted one-hot wv = iy*onehot66(jv) is built FACTORED: jv = 6a + b,
  wv[p, 6a+b] = onehot11(a)[a] * (iy*onehot6(b))[b].  Per pixel that is
  22 + 6 + 6 + 66 = 100 mask elements instead of 66 + 66 + 66 = 198 for
  {eq, eq, mult} direct.  The u-side one-hot stays direct (66).  All index and
  weight operands are stored as bf16 *pairs* (each value duplicated in adjacent
  columns) so broadcast access patterns keep innermost step=1 and the DVE runs
  in 2x_1P packed mode; onehot11 is built pair-duplicated (vs iota 0,0,1,1,..)
  so the combine can broadcast it over the b-dim with innermost step=1.
  H is accumulated on the tensor engine: per 128-pixel tile, H += wv^T @ mu
  into one PSUM bank across all 768 tiles (PE runs ~32ns/tile when unblocked,
  far below DVE cadence).  Host folds H (2x2 box sum), combines core pairs,
  normalizes, sqrts.
"""

import os

import numpy as np

import concourse.bacc as bacc
import concourse.tile as tile
from concourse import mybir
from concourse.bass_utils import run_bass_kernel_spmd

NBINS = 64
HIST_LO, HIST_HI = -2.85, 2.85
EPS_BIN = (HIST_HI - HIST_LO) / (NBINS - 1)
EPS = 1e-8
P = 128
T = 768  # 128*768 = 98304 pixels per core = half of one batch image
NB = 66  # one-hot width: k+1 in [0, 64] + 1 dead column (= 11*6)
NA = 11  # outer digit: j' = 6*a + b
NBB = 6  # inner digit
TC = 64  # max tiles per mask chunk
CHUNK_SIZES = [64] * 11 + [44, 20]  # split last chunk -> short serial PE tail
CHUNK_STARTS = [sum(CHUNK_SIZES[:i]) for i in range(len(CHUNK_SIZES))]
MAGIC = 1.5 * 2.0**23  # round-to-int bias; 1.5*2^23 keeps x+MAGIC in
# [2^23, 2^24) where the f32 grid is uniformly 1.0 (at 2^23 exactly, the
# grid below is 0.5 and e.g. a=0 digits round to -0.5 and get dropped)

f32 = mybir.dt.float32
bf16 = mybir.dt.bfloat16
Act = mybir.ActivationFunctionType
Alu = mybir.AluOpType

_cache = {}


def _build_bass():
    nc = bacc.Bacc("TRN2", target_bir_lowering=False, debug=False, num_devices=8)
    rgb = nc.declare_dram_parameter("rgb", [3, P, T], f32, isOutput=False)
    # host-built iota constants [iota66 | iota11 dup-pairs | iota6]
    cst = nc.declare_dram_parameter("cst", [P, NB + 2 * NA + NBB], bf16, isOutput=False)
    hist = nc.declare_dram_parameter("hist", [NB, NB], f32, isOutput=True)

    gps_chunks = {
        int(x) for x in os.environ.get("HIST_GPS_CHUNKS", "").split(",") if x != ""
    }

    with tile.TileContext(nc) as tc:
        with (
            tc.tile_pool(name="const", bufs=1) as cpool,
            tc.tile_pool(name="px", bufs=1) as px,
            tc.tile_pool(name="mask", bufs=3) as mpool,
            tc.tile_pool(name="psum", bufs=1, space="PSUM") as pp,
        ):
            r = px.tile([P, T], f32, tag="r")
            g = px.tile([P, T], f32, tag="g")
            b = px.tile([P, T], f32, tag="b")
            # r and g gate the first mask op; load them first, split for
            # queue parallelism, b last.
            H2 = T // 2
            nc.sync.dma_start(r[:, 0:H2], rgb[0, :, 0:H2])
            nc.sync.dma_start(r[:, H2:T], rgb[0, :, H2:T])
            nc.sync.dma_start(g[:, 0:H2], rgb[1, :, 0:H2])
            nc.sync.dma_start(g[:, H2:T], rgb[1, :, H2:T])
            nc.sync.dma_start(b[:, 0:H2], rgb[2, :, 0:H2])
            nc.sync.dma_start(b[:, H2:T], rgb[2, :, H2:T])

            # tiny host-built iotas (DMA'd, no gpsimd); broadcast over tiles
            # via stride-0 dims at use sites
            iotas = cpool.tile([P, NB + 2 * NA + NBB], bf16, tag="iotas")
            nc.sync.dma_start(iotas[:], cst[:])
            iota66 = iotas[:, 0:NB]
            iota11p = iotas[:, NB : NB + 2 * NA]
            iota6 = iotas[:, NB + 2 * NA :]

            # Pre-touch each DMA-produced tile with a single-input op so the
            # DVE's vector clock observes each DMA semaphore once; later
            # multi-input ops then need fewer waits.
            warm = cpool.tile([P, 4], f32, tag="warm")
            nc.vector.tensor_copy(warm[:, 0:1], r[:, 0:1])
            nc.vector.tensor_copy(warm[:, 1:2], g[:, 0:1])
            nc.vector.tensor_copy(warm[:, 2:3], b[:, 0:1])

            eps_bias = cpool.tile([P, 1], f32, tag="eps_bias")
            nc.vector.memset(eps_bias[:], EPS)
            # tiny dummy Ln preloads the ACT table before the DMA completes
            tbl_warm = cpool.tile([P, 1], f32, tag="tbl_warm")
            nc.scalar.activation(tbl_warm[:], eps_bias[:], Act.Ln, bias=eps_bias[:])
            lr = px.tile([P, T], f32, tag="lr")
            lg = px.tile([P, T], f32, tag="lg")
            lb = px.tile([P, T], f32, tag="lb")
            nc.scalar.activation(lr[:], r[:], Act.Ln, bias=eps_bias[:])
            nc.scalar.activation(lg[:], g[:], Act.Ln, bias=eps_bias[:])
            nc.scalar.activation(lb[:], b[:], Act.Ln, bias=eps_bias[:])

            u = px.tile([P, T], f32, tag="u")
            v = px.tile([P, T], f32, tag="v")
            nc.vector.tensor_tensor(u[:], lg[:], lr[:], op=Alu.subtract)
            nc.vector.tensor_tensor(v[:], lg[:], lb[:], op=Alu.subtract)

            # iu+MAGIC = u/eps_bin + (0.5 - LO/eps_bin) + MAGIC: the magic add
            # rounds to integer; the -2^23 is subtracted exactly by the ACT
            # pair-copy bias below.
            iu = px.tile([P, T], f32, tag="iu")
            jvm = px.tile([P, T], f32, tag="jvm")
            nc.vector.tensor_scalar(
                iu[:], u[:], 1.0 / EPS_BIN, 0.5 - HIST_LO / EPS_BIN + MAGIC,
                op0=Alu.mult, op1=Alu.add,
            )
            nc.vector.tensor_scalar(
                jvm[:], v[:], -1.0 / EPS_BIN, 0.5 + HIST_HI / EPS_BIN + MAGIC,
                op0=Alu.mult, op1=Alu.add,
            )
            # jv digits: jv = 6a + b.  jvs = exact small integer;
            # a = round(jv/6 - 5/12 + MAGIC) = floor(jv/6) for integer jv
            # (margins to the .5 rounding boundary are >= 1/12, product error
            # ~2^-24*|jv|/6 is negligible).  b = jv - 6a computed on de-magic'd
            # values only: anything ~6*2^23 has ulp 4 and would corrupt b.
            jvs = px.tile([P, T], f32, tag="jvs")
            nc.vector.tensor_scalar(
                jvs[:], jvm[:], 1.0, -MAGIC, op0=Alu.mult, op1=Alu.add
            )
            # MAGIC - 2.5/6 is NOT representable in f32 (ulp near 2^23 is 0.5,
            # the offset would collapse to -0.5 and round-half-even would then
            # misplace every b=0/a-odd pixel); apply the small offset first.
            a1 = px.tile([P, T], f32, tag="a1")
            nc.vector.tensor_scalar(
                a1[:], jvs[:], 1.0 / 6.0, -2.5 / 6.0, op0=Alu.mult, op1=Alu.add
            )
            am = px.tile([P, T], f32, tag="am")
            nc.vector.tensor_scalar(
                am[:], a1[:], 1.0, MAGIC, op0=Alu.mult, op1=Alu.add
            )
            asm = px.tile([P, T], f32, tag="asm")
            nc.vector.tensor_scalar(
                asm[:], am[:], 1.0, -MAGIC, op0=Alu.mult, op1=Alu.add
            )
            bsm = px.tile([P, T], f32, tag="bsm")
            nc.vector.scalar_tensor_tensor(
                bsm[:], asm[:], -6.0, jvs[:], op0=Alu.mult, op1=Alu.add
            )
            negM = cpool.tile([P, 1], f32, tag="negM")
            nc.vector.memset(negM[:], -MAGIC)
            zbias = cpool.tile([P, 1], f32, tag="zbias")
            nc.vector.memset(zbias[:], 0.0)

            # Iy = sqrt(r^2+g^2+b^2) * (r+g+b > EPS)
            r2 = px.tile([P, T], f32, tag="r2")
            g2 = px.tile([P, T], f32, tag="g2")
            b2 = px.tile([P, T], f32, tag="b2")
            nc.scalar.activation(r2[:], r[:], Act.Square)
            nc.scalar.activation(g2[:], g[:], Act.Square)
            nc.scalar.activation(b2[:], b[:], Act.Square)
            ss = px.tile([P, T], f32, tag="ss")
            nc.vector.tensor_tensor(ss[:], r2[:], g2[:], op=Alu.add)
            nc.vector.tensor_tensor(ss[:], ss[:], b2[:], op=Alu.add)
            # valid = (r+g+b > 1e-8) is omitted: with uniform [0,1) inputs the
            # probability of a pixel failing it is ~1e-24, and even then the
            # histogram perturbation would be ~1e-8 of one cell.

            # bf16 pair copies (value duplicated in adjacent columns).
            # Chunk-0 slices go into dedicated small tiles FIRST so the first
            # mask ops can issue before the full-width prep completes.
            def pair_full(name, src, bias, act=Act.Identity):
                pt0 = cpool.tile([P, 2 * TC], bf16, tag=name + "0")
                pt = px.tile([P, 2 * T], bf16, tag=name)
                nc.scalar.activation(
                    pt0[:].rearrange("p (t two) -> p two t", two=2),
                    src[:, 0:TC].unsqueeze(1).to_broadcast([P, 2, TC]),
                    act,
                    bias=bias,
                )
                nc.scalar.activation(
                    pt[:, 2 * TC :].rearrange("p (t two) -> p two t", two=2),
                    src[:, TC:].unsqueeze(1).to_broadcast([P, 2, T - TC]),
                    act,
                    bias=bias,
                )
                return pt0, pt

            iu_p0, iu_p = pair_full("iu_p", iu, negM[:])
            a_p0, a_p = pair_full("a_p", am, negM[:])
            b_p0, b_p = pair_full("b_p", bsm, zbias[:])
            iy_p0, iy_p = pair_full("iy_p", ss, zbias[:], act=Act.Sqrt)

            pairs_full = {"iu": iu_p, "a": a_p, "b": b_p, "iy": iy_p}
            pairs_0 = {"iu": iu_p0, "a": a_p0, "b": b_p0, "iy": iy_p0}

            def pair_bcast(key, c, inner):
                st, sz = CHUNK_STARTS[c], CHUNK_SIZES[c]
                full = pairs_full[key]
                if c == 0:
                    sl = pairs_0[key][:]
                else:
                    sl = full[:, st * 2 : (st + sz) * 2]
                return (
                    sl.rearrange("p (t two) -> p t two", two=2)
                    .unsqueeze(2)
                    .to_broadcast([P, sz, inner, 2])
                )

            def iota_bcast(tl, sz, inner):
                return (
                    tl.rearrange("p (h two) -> p h two", two=2)
                    .unsqueeze(1)
                    .to_broadcast([P, sz, inner, 2])
                )

            direct_v = bool(int(os.environ.get("HIST_DIRECT", "0")))
            if direct_v:
                jv_p0, jv_p = pair_full("jv_p", jvm, negM[:])
                pairs_full["jv"] = jv_p
                pairs_0["jv"] = jv_p0

            hp = pp.tile([NB, NB], f32, tag="hp")
            for c, (cst, csz) in enumerate(zip(CHUNK_STARTS, CHUNK_SIZES)):
                mu = mpool.tile([P, TC * NB], bf16, tag="mu")
                da = mpool.tile([P, TC * 2 * NA], bf16, tag="da")
                wb = mpool.tile([P, TC * NBB], bf16, tag="wb")
                wv = mpool.tile([P, TC * NB], bf16, tag="wv")
                mu4 = mu[:, 0 : csz * NB].rearrange(
                    "p (t h two) -> p t h two", h=NB // 2, two=2
                )
                da4 = da[:, 0 : csz * 2 * NA].rearrange(
                    "p (t k two) -> p t k two", k=NA, two=2
                )
                wb4 = wb[:, 0 : csz * NBB].rearrange(
                    "p (t h two) -> p t h two", h=NBB // 2, two=2
                )
                nc.vector.tensor_tensor(
                    mu4, pair_bcast("iu", c, NB // 2), iota_bcast(iota66, csz, NB // 2),
                    op=Alu.is_equal,
                )
                if direct_v:
                    wv4 = wv[:, 0 : csz * NB].rearrange(
                        "p (t h two) -> p t h two", h=NB // 2, two=2
                    )
                    nc.vector.tensor_tensor(
                        wv4, pair_bcast("jv", c, NB // 2),
                        iota_bcast(iota66, csz, NB // 2), op=Alu.is_equal,
                    )
                    nc.vector.tensor_tensor(
                        wv4, wv4, pair_bcast("iy", c, NB // 2), op=Alu.mult
                    )
                else:
                    nc.vector.tensor_tensor(
                        da4, pair_bcast("a", c, NA), iota_bcast(iota11p, csz, NA),
                        op=Alu.is_equal,
                    )
                    nc.vector.tensor_tensor(
                        wb4, pair_bcast("b", c, NBB // 2),
                        iota_bcast(iota6, csz, NBB // 2), op=Alu.is_equal,
                    )
                    nc.vector.tensor_tensor(
                        wb4, wb4, pair_bcast("iy", c, NBB // 2), op=Alu.mult
                    )
                    # wv[p, t, a, h, two] = da[p, t, a(dup-pair)] * wb[p, t, (h,two)]
                    da_e = (
                        da[:, 0 : csz * 2 * NA]
                        .rearrange("p (t a two) -> p t a two", a=NA, two=2)
                        .unsqueeze(3)
                        .to_broadcast([P, csz, NA, NBB // 2, 2])
                    )
                    wb_e = (
                        wb[:, 0 : csz * NBB]
                        .rearrange("p (t h two) -> p t h two", h=NBB // 2, two=2)
                        .unsqueeze(2)
                        .to_broadcast([P, csz, NA, NBB // 2, 2])
                    )
                    wv5 = wv[:, 0 : csz * NB].rearrange(
                        "p (t a h two) -> p t a h two", a=NA, h=NBB // 2, two=2
                    )
                    eng = nc.gpsimd if c in gps_chunks else nc.vector
                    eng.tensor_tensor(wv5, da_e, wb_e, op=Alu.mult)
                for t in range(csz):
                    gt = cst + t
                    nc.tensor.matmul(
                        hp[:],
                        lhsT=wv[:, t * NB : (t + 1) * NB],
                        rhs=mu[:, t * NB : (t + 1) * NB],
                        start=(gt == 0),
                        stop=(gt == T - 1),
                    )

            hs = cpool.tile([NB, NB], f32, tag="hs")
            nc.scalar.activation(hs[:], hp[:], Act.Copy)
            nc.sync.dma_start(hist[:], hs[:])
    nc.compile()
    return nc


def kernel(img: np.ndarray) -> np.ndarray:
    B, C, H, W_ = img.shape
    assert (B, C, H, W_) == (4, 3, 384, 512)
    img = np.ascontiguousarray(np.asarray(img, dtype=np.float32))

    if "nc" not in _cache:
        _cache["nc"] = _build_bass()
    nc = _cache["nc"]

    if "cst" not in _cache:
        import ml_dtypes

        row = np.concatenate(
            [
                np.arange(NB),
                np.repeat(np.arange(NA), 2),
                np.arange(NBB),
            ]
        ).astype(ml_dtypes.bfloat16)
        _cache["cst"] = np.ascontiguousarray(np.broadcast_to(row, (P, row.size)))
    cst = _cache["cst"]

    in_maps = []
    for core in range(8):
        bb, half = divmod(core, 2)
        shard = img[bb, :, half * 192 : (half + 1) * 192, :].reshape(3, P, T)
        in_maps.append({"rgb": np.ascontiguousarray(shard), "cst": cst})

    trace = bool(int(os.environ.get("HIST_TRACE", "0")))
    res = run_bass_kernel_spmd(nc, in_maps, list(range(8)), trace=trace)
    if trace:
        print(f"HW exec time: {res.exec_time_ns} ns")
        _cache["exec_time_ns"] = res.exec_time_ns

    out = np.empty((4, NBINS, NBINS), dtype=np.float32)
    for bb in range(4):
        h = res.results[2 * bb]["hist"].astype(np.float64) + res.results[
            2 * bb + 1
        ]["hist"].astype(np.float64)
        n = (
            h[0:64, 0:64]
            + h[0:64, 1:65]
            + h[1:65, 0:64]
            + h[1:65, 1:65]
        ) + 1e-8
        norm = n.sum() + 1e-8
        out[bb] = np.sqrt(n / norm).astype(np.float32)
    return out
